# revision 8
# baseline (speedup 1.0000x reference)
"""Anchor3DHead head (three 1x1 convs) as one fused channel-contraction
matmul, sharded over 8 TRN2 NeuronCores.

Math: for x [B, C, H, W] and W_cat = [w_cls | w_reg | w_dir] ([C, 20]),
    out[b, o, h, w] = sum_c x[b, c, h, w] * W_cat[c, o] + b_cat[o]
Each core handles one (batch, H-half) shard: rhs = x-shard [C, 26784]
streamed through the PE with W_cat chunks stationary, accumulated over
3 K-chunks of 128 channels into PSUM, bias added on the vector engine.
"""

import numpy as np

import concourse.bacc as bacc
import concourse.mybir as mybir
from concourse.bass_utils import run_bass_kernel_spmd
from concourse.tile import TileContext

B, C, H, W = 4, 384, 248, 216
O_CLS, O_REG, O_DIR = 2, 14, 4
O = O_CLS + O_REG + O_DIR      # 20
N_CORES = 8
H_SH = H // 2                  # 124 H-rows per core (4 batches x 2 H-halves)
N_SH = H_SH * W                # 26784 spatial positions per core
K_CHUNKS = C // 128            # 3

N_CHUNK = 2976                 # spatial cols per x DMA (1.52 MB per [128, N_CHUNK] tile)
N_SUB = 496                    # cols per matmul (<=512 fp32 / one PSUM bank)

MM_DTYPE = mybir.dt.float32


def build_nc(n_sh=N_SH, n_chunk=N_CHUNK, n_sub=N_SUB, mm_dtype=MM_DTYPE,
             x_bufs=3, o_bufs=3, ps_bufs=8):
    assert n_sh % n_chunk == 0 and n_chunk % n_sub == 0
    nc = bacc.Bacc(num_devices=N_CORES)
    xs = nc.dram_tensor("xs", [C, n_sh], mybir.dt.float32, kind="ExternalInput")
    wcat = nc.dram_tensor("wcat", [C, O], mybir.dt.float32, kind="ExternalInput")
    bcat = nc.dram_tensor("bcat", [O, 1], mybir.dt.float32, kind="ExternalInput")
    out = nc.dram_tensor("out", [O, n_sh], mybir.dt.float32, kind="ExternalOutput")

    with TileContext(nc) as tc:
        with tc.tile_pool(name="consts", bufs=1) as cpool, \
             tc.tile_pool(name="x", bufs=x_bufs) as xpool, \
             tc.tile_pool(name="o", bufs=o_bufs) as opool, \
             tc.tile_pool(name="ps", bufs=ps_bufs, space="PSUM") as ppool:
            # [C, n] DRAM views reshaped so all 3 K-chunks land in one DMA:
            # dest[p, k, ...] = src[k*128 + p, ...]
            xs_v = xs[:, :].rearrange("(k p) n -> p k n", k=K_CHUNKS)
            w_v = wcat[:, :].rearrange("(k p) o -> p k o", k=K_CHUNKS)

            w_sb = cpool.tile([128, K_CHUNKS, O], mybir.dt.float32)
            nc.sync.dma_start(out=w_sb, in_=w_v)
            b_sb = cpool.tile([O, 1], mybir.dt.float32)
            nc.sync.dma_start(out=b_sb, in_=bcat[:, :])

            for ci in range(n_sh // n_chunk):
                xt = xpool.tile([128, K_CHUNKS, n_chunk], mybir.dt.float32,
                                tag="xt")
                nc.sync.dma_start(
                    out=xt,
                    in_=xs_v[:, :, ci * n_chunk:(ci + 1) * n_chunk])
                ot = opool.tile([O, n_chunk], mybir.dt.float32, tag="ot")
                for si in range(n_chunk // n_sub):
                    ps = ppool.tile([O, n_sub], mybir.dt.float32, tag="ps")
                    for k in range(K_CHUNKS):
                        lhsT = w_sb[:, k, :]
                        rhs = xt[:, k, si * n_sub:(si + 1) * n_sub]
                        if mm_dtype != mybir.dt.float32:
                            lhsT = lhsT.bitcast(mm_dtype)
                            rhs = rhs.bitcast(mm_dtype)
                        nc.tensor.matmul(ps, lhsT=lhsT, rhs=rhs,
                                         start=(k == 0), stop=(k == K_CHUNKS - 1))
                    nc.vector.tensor_scalar_add(
                        ot[:, si * n_sub:(si + 1) * n_sub], ps, b_sb)
                nc.sync.dma_start(out=out[:, ci * n_chunk:(ci + 1) * n_chunk],
                                  in_=ot)
    nc.compile()
    return nc


def shard_inputs(x, w_cls, b_cls, w_reg, b_reg, w_dir, b_dir):
    wcat = np.ascontiguousarray(
        np.concatenate([w_cls, w_reg, w_dir], axis=1), dtype=np.float32)
    bcat = np.ascontiguousarray(
        np.concatenate([b_cls, b_reg, b_dir]).reshape(O, 1), dtype=np.float32)
    in_maps = []
    for i in range(N_CORES):
        b, h0 = divmod(i, 2)
        xs = np.ascontiguousarray(
            x[b, :, h0 * H_SH:(h0 + 1) * H_SH, :], dtype=np.float32
        ).reshape(C, N_SH)
        in_maps.append({"xs": xs, "wcat": wcat, "bcat": bcat})
    return in_maps


def assemble_output(results):
    full = np.empty((B, O, H, W), dtype=np.float32)
    for i in range(N_CORES):
        b, h0 = divmod(i, 2)
        full[b, :, h0 * H_SH:(h0 + 1) * H_SH, :] = \
            results[i]["out"].reshape(O, H_SH, W)
    cls_score = np.ascontiguousarray(full[:, :O_CLS])
    bbox_pred = np.ascontiguousarray(full[:, O_CLS:O_CLS + O_REG])
    dir_cls = np.ascontiguousarray(full[:, O_CLS + O_REG:])
    return cls_score, bbox_pred, dir_cls


_NC_CACHE = {}


def run(x, w_cls, b_cls, w_reg, b_reg, w_dir, b_dir, build_kwargs=None,
        **spmd_kwargs):
    """Build (cached) + run on 8 cores; returns (outputs_tuple, BassKernelResults)."""
    key = tuple(sorted((build_kwargs or {}).items()))
    if key not in _NC_CACHE:
        _NC_CACHE[key] = build_nc(**(build_kwargs or {}))
    nc = _NC_CACHE[key]
    in_maps = shard_inputs(x, w_cls, b_cls, w_reg, b_reg, w_dir, b_dir)
    res = run_bass_kernel_spmd(nc, in_maps, list(range(N_CORES)), **spmd_kwargs)
    return assemble_output(res.results), res


def kernel(x, w_cls, b_cls, w_reg, b_reg, w_dir, b_dir):
    outs, _ = run(np.asarray(x), np.asarray(w_cls), np.asarray(b_cls),
                  np.asarray(w_reg), np.asarray(b_reg),
                  np.asarray(w_dir), np.asarray(b_dir))
    return outs


# revision 12
# speedup vs baseline: 1.3175x; 1.3175x over previous
"""Anchor3DHead head (three 1x1 convs) as one fused channel-contraction
matmul, sharded over 8 TRN2 NeuronCores.

Math: for x [B, C, H, W] and W_cat = [w_cls | w_reg | w_dir] ([C, 20]),
    out[b, o, h, w] = sum_c x[b, c, h, w] * W_cat[c, o] + b_cat[o]
Each core handles one (batch, H-half) shard: rhs = x-shard [C, 26784]
streamed through the PE with W_cat chunks stationary, accumulated over
3 K-chunks of 128 channels into PSUM, bias added on the vector engine.
"""

import numpy as np

import concourse.bacc as bacc
import concourse.mybir as mybir
from concourse.bass_utils import run_bass_kernel_spmd
from concourse.tile import TileContext

B, C, H, W = 4, 384, 248, 216
O_CLS, O_REG, O_DIR = 2, 14, 4
O = O_CLS + O_REG + O_DIR      # 20
N_CORES = 8
H_SH = H // 2                  # 124 H-rows per core (4 batches x 2 H-halves)
N_SH = H_SH * W                # 26784 spatial positions per core
K_CHUNKS = C // 128            # 3

N_CHUNK = 2976                 # spatial cols per x DMA (1.52 MB per [128, N_CHUNK] tile)
N_SUB = 496                    # cols per matmul (<=512 fp32 / one PSUM bank)

MM_DTYPE = mybir.dt.float32


def build_nc(n_sh=N_SH, n_chunk=N_CHUNK, n_sub=N_SUB, mm_dtype=MM_DTYPE,
             x_bufs=3, o_bufs=3, ps_bufs=8):
    assert n_sh % n_chunk == 0 and n_chunk % n_sub == 0
    nc = bacc.Bacc(num_devices=N_CORES)
    xs = nc.dram_tensor("xs", [C, n_sh], mm_dtype, kind="ExternalInput")
    wcat = nc.dram_tensor("wcat", [C, O], mm_dtype, kind="ExternalInput")
    bcat = nc.dram_tensor("bcat", [O, 1], mybir.dt.float32, kind="ExternalInput")
    out = nc.dram_tensor("out", [O, n_sh], mybir.dt.float32, kind="ExternalOutput")

    with TileContext(nc) as tc:
        with tc.tile_pool(name="consts", bufs=1) as cpool, \
             tc.tile_pool(name="x", bufs=x_bufs) as xpool, \
             tc.tile_pool(name="o", bufs=o_bufs) as opool, \
             tc.tile_pool(name="ps", bufs=ps_bufs, space="PSUM") as ppool:
            # [C, n] DRAM views reshaped so all 3 K-chunks land in one DMA:
            # dest[p, k, ...] = src[k*128 + p, ...]
            xs_v = xs[:, :].rearrange("(k p) n -> p k n", k=K_CHUNKS)
            w_v = wcat[:, :].rearrange("(k p) o -> p k o", k=K_CHUNKS)

            w_sb = cpool.tile([128, K_CHUNKS, O], mm_dtype)
            nc.sync.dma_start(out=w_sb, in_=w_v)
            b_sb = cpool.tile([O, 1], mybir.dt.float32)
            nc.sync.dma_start(out=b_sb, in_=bcat[:, :])

            for ci in range(n_sh // n_chunk):
                xt = xpool.tile([128, K_CHUNKS, n_chunk], mm_dtype, tag="xt")
                nc.sync.dma_start(
                    out=xt,
                    in_=xs_v[:, :, ci * n_chunk:(ci + 1) * n_chunk])
                ot = opool.tile([O, n_chunk], mybir.dt.float32, tag="ot")
                for si in range(n_chunk // n_sub):
                    ps = ppool.tile([O, n_sub], mybir.dt.float32, tag="ps")
                    for k in range(K_CHUNKS):
                        nc.tensor.matmul(ps, lhsT=w_sb[:, k, :],
                                         rhs=xt[:, k, si * n_sub:(si + 1) * n_sub],
                                         start=(k == 0), stop=(k == K_CHUNKS - 1))
                    nc.vector.tensor_scalar_add(
                        ot[:, si * n_sub:(si + 1) * n_sub], ps, b_sb)
                nc.sync.dma_start(out=out[:, ci * n_chunk:(ci + 1) * n_chunk],
                                  in_=ot)
    nc.compile()
    return nc


def shard_inputs(x, w_cls, b_cls, w_reg, b_reg, w_dir, b_dir):
    wcat = np.ascontiguousarray(
        np.concatenate([w_cls, w_reg, w_dir], axis=1), dtype=np.float32)
    bcat = np.ascontiguousarray(
        np.concatenate([b_cls, b_reg, b_dir]).reshape(O, 1), dtype=np.float32)
    in_maps = []
    for i in range(N_CORES):
        b, h0 = divmod(i, 2)
        xs = np.ascontiguousarray(
            x[b, :, h0 * H_SH:(h0 + 1) * H_SH, :], dtype=np.float32
        ).reshape(C, N_SH)
        in_maps.append({"xs": xs, "wcat": wcat, "bcat": bcat})
    return in_maps


def assemble_output(results):
    full = np.empty((B, O, H, W), dtype=np.float32)
    for i in range(N_CORES):
        b, h0 = divmod(i, 2)
        full[b, :, h0 * H_SH:(h0 + 1) * H_SH, :] = \
            results[i]["out"].reshape(O, H_SH, W)
    cls_score = np.ascontiguousarray(full[:, :O_CLS])
    bbox_pred = np.ascontiguousarray(full[:, O_CLS:O_CLS + O_REG])
    dir_cls = np.ascontiguousarray(full[:, O_CLS + O_REG:])
    return cls_score, bbox_pred, dir_cls


_NC_CACHE = {}


def run(x, w_cls, b_cls, w_reg, b_reg, w_dir, b_dir, build_kwargs=None,
        **spmd_kwargs):
    """Build (cached) + run on 8 cores; returns (outputs_tuple, BassKernelResults)."""
    key = tuple(sorted((build_kwargs or {}).items()))
    if key not in _NC_CACHE:
        _NC_CACHE[key] = build_nc(**(build_kwargs or {}))
    nc = _NC_CACHE[key]
    in_maps = shard_inputs(x, w_cls, b_cls, w_reg, b_reg, w_dir, b_dir)
    res = run_bass_kernel_spmd(nc, in_maps, list(range(N_CORES)), **spmd_kwargs)
    return assemble_output(res.results), res


def kernel(x, w_cls, b_cls, w_reg, b_reg, w_dir, b_dir):
    outs, _ = run(np.asarray(x), np.asarray(w_cls), np.asarray(b_cls),
                  np.asarray(w_reg), np.asarray(b_reg),
                  np.asarray(w_dir), np.asarray(b_dir))
    return outs
